# revision 47
# baseline (speedup 1.0000x reference)
"""Trainium2 Bass kernel for causal self-attention with segment masking.

Sharding: 8 cores = 2 batches x 4 head-groups (4 heads each).
Per core: QKV projection (bf16), S^T-layout attention with data-dependent
tile skipping AND per-tile q-column windowing (causal + segment structure),
output projection producing a partial [T, D] sum; host adds the 4 partials
per batch.

Layouts (per core):
  x_T   [D, T]      bf16  (host-transposed)
  q_T/k_T [128, T]  bf16  two tiles, one per head pair (2 heads x 64 dims)
  v_ext [128, 16kb, 4h, 65] bf16 (col 64 = ones -> softmax denominator)
  s     [128k, 2h, 512q] f32 PSUM, written only on the tile's live window
  pt    [128, 2, 512] bf16 SBUF = exp(s/8) * mask01 (window only)
  y_ps  [65, 512]   f32 PSUM = v_ext.T @ pt (row 64 = sum of p = denom);
        ragged window accumulation relies on per-element has_written bits
  y_qc  [128, 2, T] bf16 (normalized, feeds proj as lhsT)

Scheduling notes (the performance-critical part):
  - Every engine queue executes strictly in order, so the softmax
    normalization chain (y PSUM->SBUF evac, denom-row gather DMA, DVE
    reciprocal, gather-back DMA, broadcast, scaling multiplies) is emitted
    in pipeline stages spread over the next two attention groups; each op
    is emitted at least one full group after its producer.
  - 1/denom is broadcast over partitions with rank-1 PE matmuls
    (ones[1,64]^T @ r[1,512]); head 1 lands on partitions 64-127 via the
    col tile_position, and its unnormalized y is partition-shifted a full
    group early, so the final multiplies write y_qc in place.
  - A short junk-matmul burn after the first weight-chunk DMA opens the
    HAM clock gate (PE idles at 1.2 GHz otherwise) before real work.
  - PSUM budget (8 banks): psq 2 (qkv/proj chains) + pss 4 (score tiles)
    + psy 2 (y / broadcast tiles).
"""

import numpy as np
import ml_dtypes

import concourse.bass as bass
import concourse.mybir as mybir
import concourse.tile as tile
from concourse import bacc
from concourse import bass_utils

B, T, D = 2, 2048, 1024
H, HD = 16, 64
QC = 512            # q chunk (max matmul free dim)
KB = 128            # k block (partition dim)
NQC = T // QC       # 4
NKB = T // KB       # 16
DK = D // 128       # 8 contraction chunks for projections
BF16 = mybir.dt.bfloat16
F32 = mybir.dt.float32
nbf = ml_dtypes.bfloat16
Exp = mybir.ActivationFunctionType.Exp


def _schedule(seg):
    """Data-dependent tile schedule, shared (union) across both batches.

    Returns (act, mask_arrs, wtot):
      act: {qc: [(kb, w0, w1, moff)]} where [w0, w1) is the live q-column
           window within the chunk and moff the column offset of this
           tile's mask in the packed mask tensor (-1 = all-ones window).
      mask_arrs: per-batch packed bf16 {0,1} masks [KB, wtot].
    """
    ar = np.arange(T)
    masks = [
        (seg[b][:, None] == seg[b][None, :]) & (ar[:, None] <= ar[None, :])
        for b in range(B)
    ]  # mask_T[k, q]
    act = {qc: [] for qc in range(NQC)}
    mask_cols = [[] for _ in range(B)]
    wtot = 0
    for qc in range(NQC):
        for kb in range(NKB):
            if kb * KB > qc * QC + QC - 1:
                continue  # fully above the diagonal
            subs = [
                masks[b][kb * KB:(kb + 1) * KB, qc * QC:(qc + 1) * QC]
                for b in range(B)
            ]
            u = subs[0] | subs[1]
            if not u.any():
                continue  # dead tile in both batches: skip entirely
            idx = np.nonzero(u.any(axis=0))[0]
            w0 = int(idx[0]) & ~3
            w1 = min(QC, (int(idx[-1]) + 4) & ~3)
            win = [s[:, w0:w1] for s in subs]
            if all(w.all() for w in win):
                act[qc].append((kb, w0, w1, -1))
            else:
                act[qc].append((kb, w0, w1, wtot))
                for b in range(B):
                    mask_cols[b].append(win[b].astype(nbf))
                wtot += w1 - w0
    if wtot == 0:
        wtot = 4
        mask_arrs = [np.zeros((KB, 4), nbf) for _ in range(B)]
    else:
        mask_arrs = [
            np.ascontiguousarray(np.concatenate(mask_cols[b], axis=1))
            for b in range(B)
        ]
    return act, mask_arrs, wtot


def _build(act, wtot):
    nc = bacc.Bacc("TRN2", target_bir_lowering=False, debug=False, num_devices=8)
    xT = nc.dram_tensor("xT", [D, T], BF16, kind="ExternalInput").ap()
    wqkv = nc.dram_tensor("wqkv", [D, 768], BF16, kind="ExternalInput").ap()
    wp = nc.dram_tensor("wp", [256, D], BF16, kind="ExternalInput").ap()
    mk = nc.dram_tensor("mask", [KB, wtot], BF16, kind="ExternalInput").ap()
    out = nc.dram_tensor("out", [T, D], BF16, kind="ExternalOutput").ap()

    with tile.TileContext(nc) as tc:
        with (
            tc.tile_pool(name="const", bufs=1) as cpool,
            tc.tile_pool(name="ptp", bufs=3) as ppool,
            tc.tile_pool(name="otp", bufs=3) as opool,
            tc.tile_pool(name="nrm", bufs=3) as npool,
            tc.tile_pool(name="psq", bufs=2, space="PSUM") as psq,
            tc.tile_pool(name="pss", bufs=2, space="PSUM") as pss,
            tc.tile_pool(name="psy", bufs=2, space="PSUM") as psy,
        ):
            # constant junk tile: lets PE warm-up matmuls start before any
            # DMA lands (memset has no dependencies)
            junk_sb = cpool.tile([128, 512], BF16, tag="junk")
            nc.vector.memset(junk_sb[:], 1.0)

            # ---- input DMAs (sync HWDGE queue, batched; priority order:
            # wqkv, x[qc0], mask, wp, then x[qc1..3]) ----
            wqkv_sb = cpool.tile([128, DK, 768], BF16, tag="wqkv")
            x_sb = cpool.tile([128, DK, T], BF16, tag="x")
            nc.sync.dma_start(wqkv_sb[:, 0, :], wqkv[0:128, :])
            nc.sync.dma_start(
                x_sb[:, 0:4, 0:QC],
                xT[0:512, 0:QC].rearrange("(i p) t -> p i t", p=128))
            nc.sync.dma_start(
                wqkv_sb[:, 1:DK, :],
                wqkv[128:D, :].rearrange("(i p) n -> p i n", p=128))
            nc.sync.dma_start(
                x_sb[:, 4:DK, 0:QC],
                xT[512:D, 0:QC].rearrange("(i p) t -> p i t", p=128))
            # masks for the first q-chunk's tiles (a prefix of the packed
            # mask tensor) land before x[qc1..3] so attn(0,*) never waits
            msplit = max((m + (w1 - w0) for (_, w0, w1, m) in act[0] if m >= 0), default=0)
            mask_sb = cpool.tile([128, wtot], BF16, tag="m")
            if msplit > 0:
                nc.sync.dma_start(mask_sb[:, 0:msplit], mk[:, 0:msplit])
            def x_chunk(qc):
                nc.sync.dma_start(
                    x_sb[:, :, qc * QC:(qc + 1) * QC],
                    xT[:, qc * QC:(qc + 1) * QC].rearrange("(i p) t -> p i t", p=128))
            x_chunk(1)
            if msplit < wtot:
                nc.sync.dma_start(mask_sb[:, msplit:wtot], mk[:, msplit:wtot])
            x_chunk(2)
            wp_sb = cpool.tile([128, 2, D], BF16, tag="wp")
            nc.sync.dma_start(wp_sb[:], wp.rearrange("(c p) n -> p c n", p=128))
            x_chunk(3)

            q_sb = [cpool.tile([128, T], BF16, tag=f"q{p}", name=f"q{p}") for p in range(2)]
            k_sb = [cpool.tile([128, T], BF16, tag=f"k{p}", name=f"k{p}") for p in range(2)]
            v_sb = cpool.tile([128, NKB, 4, 65], BF16, tag="v")
            y_qc = [cpool.tile([128, 2, QC], BF16, tag=f"y{qc}", name=f"y{qc}") for qc in range(NQC)]
            nc.vector.memset(v_sb[:, :, :, 64], 1.0)
            ones_sb = cpool.tile([1, 64], BF16, tag="ones")
            nc.vector.memset(ones_sb[:], 1.0)
            # trigger the exp table-set load (~2.7us) before attention needs it
            tw = npool.tile([1, 64], BF16, tag="tw")
            nc.scalar.activation(tw[:], junk_sb[0:1, 0:64], Exp, scale=1.0)

            # PE warm-up / keep-warm burn: dependency-free junk matmuls.
            # At kernel start they open the HAM clock-gate while input DMAs
            # land; at the tail they bridge sub-2us gaps so the PE clock
            # never re-throttles to 1.2 GHz.
            _burn_n = [0]

            def emit_burn(n):
                _burn_n[0] += 1
                bt = psq.tile([128, 512], F32, tag="psq", name=f"burn{_burn_n[0]}")
                for _ in range(n):
                    nc.tensor.matmul(
                        bt[:], junk_sb[:, 0:128], junk_sb[:],
                        start=True, stop=True,
                    )

            emit_burn(28)

            # ---- building blocks ----
            def emit_qkv_qk(qc):
                for p in range(2):
                    ps = psq.tile([128, 512], F32, tag="psq", name=f"q_{qc}_{p}")
                    for i in range(DK):
                        nc.tensor.matmul(
                            ps[:], wqkv_sb[:, i, p * 128:(p + 1) * 128],
                            x_sb[:, i, qc * 512:(qc + 1) * 512],
                            start=(i == 0), stop=(i == DK - 1),
                        )
                    nc.vector.tensor_copy(out=q_sb[p][:, qc * 512:(qc + 1) * 512], in_=ps[:])
                for p in range(2):
                    ps = psq.tile([128, 512], F32, tag="psq", name=f"k_{qc}_{p}")
                    for i in range(DK):
                        nc.tensor.matmul(
                            ps[:], wqkv_sb[:, i, 256 + p * 128:256 + (p + 1) * 128],
                            x_sb[:, i, qc * 512:(qc + 1) * 512],
                            start=(i == 0), stop=(i == DK - 1),
                        )
                    nc.vector.tensor_copy(out=k_sb[p][:, qc * 512:(qc + 1) * 512], in_=ps[:])

            def emit_qkv_v(qc):
                for kb in range(qc * 4, qc * 4 + 4):
                    ps = psq.tile([128, 512], F32, tag="psq", name=f"v_{kb}")
                    for i in range(DK):
                        nc.tensor.matmul(
                            ps[:, 0:256], x_sb[:, i, kb * 128:(kb + 1) * 128],
                            wqkv_sb[:, i, 512:768],
                            start=(i == 0), stop=(i == DK - 1),
                        )
                    nc.vector.tensor_copy(
                        out=v_sb[:, kb, :, 0:64],
                        in_=ps[:, 0:256].rearrange("p (h d) -> p h d", h=4),
                    )

            def emit_attn(qc, p):
                kbs = act[qc]
                y_ps = [psy.tile([128, 512], F32, tag="psy", name=f"yps{p}_{qc}_{hh}") for hh in range(2)]
                for idx, (kb, w0, w1, moff) in enumerate(kbs):
                    w = w1 - w0
                    first, last = idx == 0, idx == len(kbs) - 1
                    s_ps = pss.tile([128, 2, 512], F32, tag="pss", name=f"s_{p}_{qc}_{kb}")
                    for hh in range(2):
                        lo = hh * 64
                        nc.tensor.matmul(
                            s_ps[:, hh, w0:w1],
                            k_sb[p][lo:lo + 64, kb * 128:(kb + 1) * 128],
                            q_sb[p][lo:lo + 64, qc * 512 + w0:qc * 512 + w1],
                            start=True, stop=True,
                        )
                    pt = ppool.tile([128, 2, 512], BF16, tag="pt", name=f"pt{p}_{qc}_{kb}")
                    nc.scalar.activation(pt[:, :, w0:w1], s_ps[:, :, w0:w1], Exp, scale=0.125)
                    if moff >= 0:
                        nc.vector.tensor_tensor(
                            out=pt[:, :, w0:w1],
                            in0=pt[:, :, w0:w1],
                            in1=mask_sb[:, None, moff:moff + w].to_broadcast((128, 2, w)),
                            op=mybir.AluOpType.mult,
                        )
                    for hh in range(2):
                        nc.tensor.matmul(
                            y_ps[hh][0:65, w0:w1], v_sb[:, kb, p * 2 + hh, :],
                            pt[:, hh, w0:w1],
                            start=first, stop=last, skip_group_check=True,
                        )
                # evacuate y to SBUF fast (releases the PSUM banks); the
                # normalization chain is emitted in stages spread over the
                # next two groups so no queue ever stalls mid-chain.
                y_sb = npool.tile([65, 2, 512], F32, tag="ysb", name=f"ysb{qc}_{p}")
                nc.scalar.copy(out=y_sb[:, 0, :], in_=y_ps[0][0:65, :])
                nc.scalar.copy(out=y_sb[:, 1, :], in_=y_ps[1][0:65, :])
                return {"qc": qc, "p": p, "y_sb": y_sb}

            # normalization pipeline stages (row 64 of y_sb = denominators)
            def emit_lp(n):
                n["lp"] = npool.tile([128, 8], F32, tag="lp", name=f"lp{n['qc']}_{n['p']}")
                nc.sync.dma_start(n["lp"][:], n["y_sb"][64:65, :, :])
                # shift head-1's unnormalized y to partitions 64-127 now,
                # a full group before the scaling multiply needs it
                n["ysh"] = npool.tile([128, 512], F32, tag="ysh", name=f"ysh{n['qc']}_{n['p']}")
                nc.sync.dma_start(n["ysh"][64:128, :], n["y_sb"][0:64, 1, :])

            def emit_recip(n):
                lpb = npool.tile([128, 8], BF16, tag="lpb", name=f"lpb{n['qc']}_{n['p']}")
                with nc.allow_low_precision(reason="bf16 softmax denominators"):
                    nc.vector.reciprocal(lpb[:], n["lp"][:])
                n["l0"] = npool.tile([1, 2, 512], BF16, tag="l0", name=f"l0{n['qc']}_{n['p']}")
                nc.sync.dma_start(n["l0"][:], lpb[:])

            def emit_finish(n):
                # broadcast 1/denom over the partitions with rank-1 matmuls
                # (head 1 lands on partitions 64-127 via col tile_position),
                # then scale y straight into the projection's lhsT layout
                qc, p, y_sb = n["qc"], n["p"], n["y_sb"]
                lb = psy.tile([128, 512], F32, tag="psy", name=f"lb{qc}_{p}")
                nc.tensor.matmul(
                    lb[0:64, :], ones_sb[:], n["l0"][:, 0, :],
                    start=True, stop=True,
                )
                nc.tensor.matmul(
                    lb[64:128, :], ones_sb[:], n["l0"][:, 1, :],
                    start=True, stop=True, skip_group_check=True,
                )
                nc.vector.tensor_mul(
                    out=y_qc[qc][0:64, p, :], in0=y_sb[0:64, 0, :],
                    in1=lb[0:64, :])
                nc.vector.tensor_mul(
                    out=y_qc[qc][64:128, p, :], in0=n["ysh"][64:128, :],
                    in1=lb[64:128, :])

            def emit_proj(qc, mts=None, tail=False):
                for mt in (mts if mts is not None else range(qc * 4, qc * 4 + 4)):
                    ot = opool.tile([128, 1024], BF16, tag="ot", name=f"ot{mt}")
                    if tail:
                        # attention is done; reuse the idle score-PSUM pool
                        # for deeper proj double-buffering
                        pst = pss.tile([128, 2, 512], F32, tag="pss", name=f"pso{mt}")
                        pslices = [pst[:, 0, :], pst[:, 1, :]]
                    else:
                        pslices = [
                            psq.tile([128, 512], F32, tag="psq", name=f"pso{mt}_{n}")[:]
                            for n in range(2)
                        ]
                    for n in range(2):
                        ps = pslices[n]
                        for c in range(2):
                            nc.tensor.matmul(
                                ps, y_qc[qc][:, c, (mt % 4) * 128:(mt % 4) * 128 + 128],
                                wp_sb[:, c, n * 512:(n + 1) * 512],
                                start=(c == 0), stop=(c == 1),
                            )
                        if n == 0:
                            nc.vector.tensor_copy(out=ot[:, 0:512], in_=ps)
                        else:
                            nc.scalar.copy(out=ot[:, 512:1024], in_=ps)
                        nc.gpsimd.dma_start(
                            out[mt * 128:(mt + 1) * 128, n * 512:(n + 1) * 512],
                            ot[:, n * 512:(n + 1) * 512])

            # ---- schedule ----
            # Normalization of group g is pipelined across the next two
            # attention groups: lp-dma before T(g+1); recip+l0 after T(g+1);
            # broadcast-matmul, scaling multiplies and the partition-shift
            # DMA after T(g+2). Every op is emitted at least a full group
            # after its producer, so no engine queue FIFO-blocks mid-chain.
            emit_qkv_qk(0)
            emit_qkv_v(0)
            n0 = emit_attn(0, 0)
            emit_qkv_qk(1)
            emit_lp(n0)
            n1 = emit_attn(0, 1)
            emit_recip(n0)
            emit_qkv_v(1)
            emit_lp(n1)
            n2 = emit_attn(1, 0)
            emit_recip(n1)
            emit_finish(n0)
            emit_qkv_qk(2)
            emit_lp(n2)
            n3 = emit_attn(1, 1)
            emit_recip(n2)
            emit_finish(n1)
            emit_qkv_v(2)
            emit_lp(n3)
            emit_proj(0)
            n4 = emit_attn(2, 0)
            emit_recip(n3)
            emit_finish(n2)
            emit_qkv_qk(3)
            emit_lp(n4)
            n5 = emit_attn(2, 1)
            emit_recip(n4)
            emit_finish(n3)
            emit_qkv_v(3)
            emit_lp(n5)
            emit_proj(1)
            n6 = emit_attn(3, 0)
            emit_recip(n5)
            emit_finish(n4)
            emit_lp(n6)
            n7 = emit_attn(3, 1)
            emit_burn(7)
            emit_lp(n7)
            emit_recip(n6)
            emit_finish(n5)
            emit_proj(2, mts=[8, 9])
            emit_burn(7)
            emit_recip(n7)
            emit_finish(n6)
            emit_proj(2, mts=[10, 11])
            emit_burn(7)
            emit_finish(n7)
            emit_proj(3)

    nc.compile()
    return nc


def _in_maps(x, seg, Wqkv, Wproj, mask_arrs):
    maps = []
    for c in range(8):
        b, g = divmod(c, 4)
        h0 = g * 4
        cs, ce = h0 * 64, h0 * 64 + 256
        maps.append({
            "xT": np.ascontiguousarray(x[b].T).astype(nbf),
            "wqkv": np.ascontiguousarray(np.concatenate(
                [Wqkv[:, cs:ce], Wqkv[:, D + cs:D + ce], Wqkv[:, 2 * D + cs:2 * D + ce]],
                axis=1)).astype(nbf),
            "wp": np.ascontiguousarray(Wproj[cs:ce, :]).astype(nbf),
            "mask": mask_arrs[b],
        })
    return maps


_CACHE = {}


def _prepare(x, segment_ids, W_qkv, W_proj):
    x = np.asarray(x, np.float32)
    seg = np.asarray(segment_ids)
    Wqkv = np.asarray(W_qkv, np.float32)
    Wproj = np.asarray(W_proj, np.float32)
    tiles, mask_arrs, wtot = _schedule(seg)
    key = (tuple((qc, t) for qc in tiles for t in tiles[qc]), wtot)
    if key not in _CACHE:
        _CACHE[key] = _build(tiles, wtot)
    nc = _CACHE[key]
    return nc, _in_maps(x, seg, Wqkv, Wproj, mask_arrs)


def kernel(x, segment_ids, W_qkv, W_proj):
    nc, in_maps = _prepare(x, segment_ids, W_qkv, W_proj)
    res = bass_utils.run_bass_kernel_spmd(nc, in_maps, core_ids=list(range(8)))
    out = np.zeros((B, T, D), np.float32)
    for c in range(8):
        out[c // 4] += res.results[c]["out"].astype(np.float32)
    return out


# revision 49
# speedup vs baseline: 1.0052x; 1.0052x over previous
"""Trainium2 Bass kernel for causal self-attention with segment masking.

Sharding: 8 cores = 2 batches x 4 head-groups (4 heads each).
Per core: QKV projection (bf16), S^T-layout attention with data-dependent
tile skipping AND per-tile q-column windowing (causal + segment structure),
output projection producing a partial [T, D] sum; host adds the 4 partials
per batch.

Layouts (per core):
  x_T   [D, T]      bf16  (host-transposed)
  q_T/k_T [128, T]  bf16  two tiles, one per head pair (2 heads x 64 dims)
  v_ext [128, 16kb, 4h, 65] bf16 (col 64 = ones -> softmax denominator)
  s     [128k, 2h, 512q] f32 PSUM, written only on the tile's live window
  pt    [128, 2, 512] bf16 SBUF = exp(s/8) * mask01 (window only)
  y_ps  [65, 512]   f32 PSUM = v_ext.T @ pt (row 64 = sum of p = denom);
        ragged window accumulation relies on per-element has_written bits
  y_qc  [128, 2, T] bf16 (normalized, feeds proj as lhsT)

Scheduling notes (the performance-critical part):
  - Every engine queue executes strictly in order, so the softmax
    normalization chain (y PSUM->SBUF evac, denom-row gather DMA, DVE
    reciprocal, gather-back DMA, broadcast, scaling multiplies) is emitted
    in pipeline stages spread over the next two attention groups; each op
    is emitted at least one full group after its producer.
  - 1/denom is broadcast over partitions with rank-1 PE matmuls
    (ones[1,64]^T @ r[1,512]); head 1 lands on partitions 64-127 via the
    col tile_position, and its unnormalized y is partition-shifted a full
    group early, so the final multiplies write y_qc in place.
  - A short junk-matmul burn after the first weight-chunk DMA opens the
    HAM clock gate (PE idles at 1.2 GHz otherwise) before real work.
  - PSUM budget (8 banks): psq 2 (qkv/proj chains) + pss 4 (score tiles)
    + psy 2 (y / broadcast tiles).
"""

import numpy as np
import ml_dtypes

import concourse.bass as bass
import concourse.mybir as mybir
import concourse.tile as tile
from concourse import bacc
from concourse import bass_utils

B, T, D = 2, 2048, 1024
H, HD = 16, 64
QC = 512            # q chunk (max matmul free dim)
KB = 128            # k block (partition dim)
NQC = T // QC       # 4
NKB = T // KB       # 16
DK = D // 128       # 8 contraction chunks for projections
BF16 = mybir.dt.bfloat16
F32 = mybir.dt.float32
nbf = ml_dtypes.bfloat16
Exp = mybir.ActivationFunctionType.Exp


def _schedule(seg):
    """Data-dependent tile schedule, shared (union) across both batches.

    Returns (act, mask_arrs, wtot):
      act: {qc: [(kb, w0, w1, moff)]} where [w0, w1) is the live q-column
           window within the chunk and moff the column offset of this
           tile's mask in the packed mask tensor (-1 = all-ones window).
      mask_arrs: per-batch packed bf16 {0,1} masks [KB, wtot].
    """
    ar = np.arange(T)
    masks = [
        (seg[b][:, None] == seg[b][None, :]) & (ar[:, None] <= ar[None, :])
        for b in range(B)
    ]  # mask_T[k, q]
    act = {qc: [] for qc in range(NQC)}
    mask_cols = [[] for _ in range(B)]
    wtot = 0
    for qc in range(NQC):
        for kb in range(NKB):
            if kb * KB > qc * QC + QC - 1:
                continue  # fully above the diagonal
            subs = [
                masks[b][kb * KB:(kb + 1) * KB, qc * QC:(qc + 1) * QC]
                for b in range(B)
            ]
            u = subs[0] | subs[1]
            if not u.any():
                continue  # dead tile in both batches: skip entirely
            idx = np.nonzero(u.any(axis=0))[0]
            w0 = int(idx[0]) & ~3
            w1 = min(QC, (int(idx[-1]) + 4) & ~3)
            win = [s[:, w0:w1] for s in subs]
            if all(w.all() for w in win):
                act[qc].append((kb, w0, w1, -1))
            else:
                act[qc].append((kb, w0, w1, wtot))
                for b in range(B):
                    mask_cols[b].append(win[b].astype(nbf))
                wtot += w1 - w0
    if wtot == 0:
        wtot = 4
        mask_arrs = [np.zeros((KB, 4), nbf) for _ in range(B)]
    else:
        mask_arrs = [
            np.ascontiguousarray(np.concatenate(mask_cols[b], axis=1))
            for b in range(B)
        ]
    return act, mask_arrs, wtot


def _build(act, wtot):
    nc = bacc.Bacc("TRN2", target_bir_lowering=False, debug=False, num_devices=8)
    xT = nc.dram_tensor("xT", [D, T], BF16, kind="ExternalInput").ap()
    wqkv = nc.dram_tensor("wqkv", [D, 768], BF16, kind="ExternalInput").ap()
    wp = nc.dram_tensor("wp", [256, D], BF16, kind="ExternalInput").ap()
    mk = nc.dram_tensor("mask", [KB, wtot], BF16, kind="ExternalInput").ap()
    out = nc.dram_tensor("out", [T, D], BF16, kind="ExternalOutput").ap()

    with tile.TileContext(nc) as tc:
        with (
            tc.tile_pool(name="const", bufs=1) as cpool,
            tc.tile_pool(name="ptp", bufs=3) as ppool,
            tc.tile_pool(name="otp", bufs=3) as opool,
            tc.tile_pool(name="nrm", bufs=3) as npool,
            tc.tile_pool(name="psq", bufs=2, space="PSUM") as psq,
            tc.tile_pool(name="pss", bufs=2, space="PSUM") as pss,
            tc.tile_pool(name="psy", bufs=2, space="PSUM") as psy,
        ):
            # constant junk tile: lets PE warm-up matmuls start before any
            # DMA lands (memset has no dependencies)
            junk_sb = cpool.tile([128, 512], BF16, tag="junk")
            nc.vector.memset(junk_sb[:], 1.0)

            # ---- input DMAs (sync HWDGE queue, batched; priority order:
            # wqkv, x[qc0], mask, wp, then x[qc1..3]) ----
            wqkv_sb = cpool.tile([128, DK, 768], BF16, tag="wqkv")
            x_sb = cpool.tile([128, DK, T], BF16, tag="x")
            nc.sync.dma_start(wqkv_sb[:, 0, :], wqkv[0:128, :])
            nc.sync.dma_start(
                x_sb[:, 0:4, 0:QC],
                xT[0:512, 0:QC].rearrange("(i p) t -> p i t", p=128))
            nc.sync.dma_start(
                wqkv_sb[:, 1:DK, :],
                wqkv[128:D, :].rearrange("(i p) n -> p i n", p=128))
            nc.sync.dma_start(
                x_sb[:, 4:DK, 0:QC],
                xT[512:D, 0:QC].rearrange("(i p) t -> p i t", p=128))
            # masks for the first q-chunk's tiles (a prefix of the packed
            # mask tensor) land before x[qc1..3] so attn(0,*) never waits
            msplit = max((m + (w1 - w0) for (_, w0, w1, m) in act[0] if m >= 0), default=0)
            mask_sb = cpool.tile([128, wtot], BF16, tag="m")
            if msplit > 0:
                nc.sync.dma_start(mask_sb[:, 0:msplit], mk[:, 0:msplit])
            def x_chunk(qc):
                nc.sync.dma_start(
                    x_sb[:, :, qc * QC:(qc + 1) * QC],
                    xT[:, qc * QC:(qc + 1) * QC].rearrange("(i p) t -> p i t", p=128))
            x_chunk(1)
            if msplit < wtot:
                nc.sync.dma_start(mask_sb[:, msplit:wtot], mk[:, msplit:wtot])
            x_chunk(2)
            wp_sb = cpool.tile([128, 2, D], BF16, tag="wp")
            nc.sync.dma_start(wp_sb[:], wp.rearrange("(c p) n -> p c n", p=128))
            x_chunk(3)

            q_sb = [cpool.tile([128, T], BF16, tag=f"q{p}", name=f"q{p}") for p in range(2)]
            k_sb = [cpool.tile([128, T], BF16, tag=f"k{p}", name=f"k{p}") for p in range(2)]
            v_sb = cpool.tile([128, NKB, 4, 65], BF16, tag="v")
            y_qc = [cpool.tile([128, 2, QC], BF16, tag=f"y{qc}", name=f"y{qc}") for qc in range(NQC)]
            nc.vector.memset(v_sb[:, :, :, 64], 1.0)
            ones_sb = cpool.tile([1, 64], BF16, tag="ones")
            nc.vector.memset(ones_sb[:], 1.0)
            # trigger the exp table-set load (~2.7us) before attention needs it
            tw = npool.tile([1, 64], BF16, tag="tw")
            nc.scalar.activation(tw[:], junk_sb[0:1, 0:64], Exp, scale=1.0)

            # PE warm-up / keep-warm burn: dependency-free junk matmuls.
            # At kernel start they open the HAM clock-gate while input DMAs
            # land; at the tail they bridge sub-2us gaps so the PE clock
            # never re-throttles to 1.2 GHz.
            _burn_n = [0]

            def emit_burn(n):
                _burn_n[0] += 1
                bt = psq.tile([128, 512], F32, tag="psq", name=f"burn{_burn_n[0]}")
                for _ in range(n):
                    nc.tensor.matmul(
                        bt[:], junk_sb[:, 0:128], junk_sb[:],
                        start=True, stop=True,
                    )

            emit_burn(18)

            # ---- building blocks ----
            def emit_qkv_qk(qc):
                for p in range(2):
                    ps = psq.tile([128, 512], F32, tag="psq", name=f"q_{qc}_{p}")
                    for i in range(DK):
                        nc.tensor.matmul(
                            ps[:], wqkv_sb[:, i, p * 128:(p + 1) * 128],
                            x_sb[:, i, qc * 512:(qc + 1) * 512],
                            start=(i == 0), stop=(i == DK - 1),
                        )
                    nc.vector.tensor_copy(out=q_sb[p][:, qc * 512:(qc + 1) * 512], in_=ps[:])
                for p in range(2):
                    ps = psq.tile([128, 512], F32, tag="psq", name=f"k_{qc}_{p}")
                    for i in range(DK):
                        nc.tensor.matmul(
                            ps[:], wqkv_sb[:, i, 256 + p * 128:256 + (p + 1) * 128],
                            x_sb[:, i, qc * 512:(qc + 1) * 512],
                            start=(i == 0), stop=(i == DK - 1),
                        )
                    nc.vector.tensor_copy(out=k_sb[p][:, qc * 512:(qc + 1) * 512], in_=ps[:])

            def emit_qkv_v(qc):
                for kb in range(qc * 4, qc * 4 + 4):
                    ps = psq.tile([128, 512], F32, tag="psq", name=f"v_{kb}")
                    for i in range(DK):
                        nc.tensor.matmul(
                            ps[:, 0:256], x_sb[:, i, kb * 128:(kb + 1) * 128],
                            wqkv_sb[:, i, 512:768],
                            start=(i == 0), stop=(i == DK - 1),
                        )
                    nc.vector.tensor_copy(
                        out=v_sb[:, kb, :, 0:64],
                        in_=ps[:, 0:256].rearrange("p (h d) -> p h d", h=4),
                    )

            def emit_attn(qc, p):
                kbs = act[qc]
                y_ps = [psy.tile([128, 512], F32, tag="psy", name=f"yps{p}_{qc}_{hh}") for hh in range(2)]
                for idx, (kb, w0, w1, moff) in enumerate(kbs):
                    w = w1 - w0
                    first, last = idx == 0, idx == len(kbs) - 1
                    s_ps = pss.tile([128, 2, 512], F32, tag="pss", name=f"s_{p}_{qc}_{kb}")
                    for hh in range(2):
                        lo = hh * 64
                        nc.tensor.matmul(
                            s_ps[:, hh, w0:w1],
                            k_sb[p][lo:lo + 64, kb * 128:(kb + 1) * 128],
                            q_sb[p][lo:lo + 64, qc * 512 + w0:qc * 512 + w1],
                            start=True, stop=True,
                        )
                    pt = ppool.tile([128, 2, 512], BF16, tag="pt", name=f"pt{p}_{qc}_{kb}")
                    nc.scalar.activation(pt[:, :, w0:w1], s_ps[:, :, w0:w1], Exp, scale=0.125)
                    if moff >= 0:
                        nc.vector.tensor_tensor(
                            out=pt[:, :, w0:w1],
                            in0=pt[:, :, w0:w1],
                            in1=mask_sb[:, None, moff:moff + w].to_broadcast((128, 2, w)),
                            op=mybir.AluOpType.mult,
                        )
                    for hh in range(2):
                        nc.tensor.matmul(
                            y_ps[hh][0:65, w0:w1], v_sb[:, kb, p * 2 + hh, :],
                            pt[:, hh, w0:w1],
                            start=first, stop=last, skip_group_check=True,
                        )
                # evacuate y to SBUF fast (releases the PSUM banks); the
                # normalization chain is emitted in stages spread over the
                # next two groups so no queue ever stalls mid-chain.
                y_sb = npool.tile([65, 2, 512], F32, tag="ysb", name=f"ysb{qc}_{p}")
                nc.scalar.copy(out=y_sb[:, 0, :], in_=y_ps[0][0:65, :])
                nc.scalar.copy(out=y_sb[:, 1, :], in_=y_ps[1][0:65, :])
                return {"qc": qc, "p": p, "y_sb": y_sb}

            # normalization pipeline stages (row 64 of y_sb = denominators)
            def emit_lp(n):
                n["lp"] = npool.tile([128, 8], F32, tag="lp", name=f"lp{n['qc']}_{n['p']}")
                nc.sync.dma_start(n["lp"][:], n["y_sb"][64:65, :, :])
                # shift head-1's unnormalized y to partitions 64-127 now,
                # a full group before the scaling multiply needs it
                n["ysh"] = npool.tile([128, 512], F32, tag="ysh", name=f"ysh{n['qc']}_{n['p']}")
                nc.sync.dma_start(n["ysh"][64:128, :], n["y_sb"][0:64, 1, :])

            def emit_recip(n):
                lpb = npool.tile([128, 8], BF16, tag="lpb", name=f"lpb{n['qc']}_{n['p']}")
                with nc.allow_low_precision(reason="bf16 softmax denominators"):
                    nc.vector.reciprocal(lpb[:], n["lp"][:])
                n["l0"] = npool.tile([1, 2, 512], BF16, tag="l0", name=f"l0{n['qc']}_{n['p']}")
                nc.sync.dma_start(n["l0"][:], lpb[:])

            def emit_finish(n):
                # broadcast 1/denom over the partitions with rank-1 matmuls
                # (head 1 lands on partitions 64-127 via col tile_position),
                # then scale y straight into the projection's lhsT layout
                qc, p, y_sb = n["qc"], n["p"], n["y_sb"]
                lb = psy.tile([128, 512], F32, tag="psy", name=f"lb{qc}_{p}")
                nc.tensor.matmul(
                    lb[0:64, :], ones_sb[:], n["l0"][:, 0, :],
                    start=True, stop=True,
                )
                nc.tensor.matmul(
                    lb[64:128, :], ones_sb[:], n["l0"][:, 1, :],
                    start=True, stop=True, skip_group_check=True,
                )
                nc.vector.tensor_mul(
                    out=y_qc[qc][0:64, p, :], in0=y_sb[0:64, 0, :],
                    in1=lb[0:64, :])
                nc.vector.tensor_mul(
                    out=y_qc[qc][64:128, p, :], in0=n["ysh"][64:128, :],
                    in1=lb[64:128, :])

            def emit_proj(qc, mts=None, tail=False):
                for mt in (mts if mts is not None else range(qc * 4, qc * 4 + 4)):
                    ot = opool.tile([128, 1024], BF16, tag="ot", name=f"ot{mt}")
                    if tail:
                        # attention is done; reuse the idle score-PSUM pool
                        # for deeper proj double-buffering
                        pst = pss.tile([128, 2, 512], F32, tag="pss", name=f"pso{mt}")
                        pslices = [pst[:, 0, :], pst[:, 1, :]]
                    else:
                        pslices = [
                            psq.tile([128, 512], F32, tag="psq", name=f"pso{mt}_{n}")[:]
                            for n in range(2)
                        ]
                    for n in range(2):
                        ps = pslices[n]
                        for c in range(2):
                            nc.tensor.matmul(
                                ps, y_qc[qc][:, c, (mt % 4) * 128:(mt % 4) * 128 + 128],
                                wp_sb[:, c, n * 512:(n + 1) * 512],
                                start=(c == 0), stop=(c == 1),
                            )
                        if n == 0:
                            nc.vector.tensor_copy(out=ot[:, 0:512], in_=ps)
                        else:
                            nc.scalar.copy(out=ot[:, 512:1024], in_=ps)
                        nc.gpsimd.dma_start(
                            out[mt * 128:(mt + 1) * 128, n * 512:(n + 1) * 512],
                            ot[:, n * 512:(n + 1) * 512])

            # ---- schedule ----
            # Normalization of group g is pipelined across the next two
            # attention groups: lp-dma before T(g+1); recip+l0 after T(g+1);
            # broadcast-matmul, scaling multiplies and the partition-shift
            # DMA after T(g+2). Every op is emitted at least a full group
            # after its producer, so no engine queue FIFO-blocks mid-chain.
            emit_qkv_qk(0)
            emit_qkv_v(0)
            n0 = emit_attn(0, 0)
            emit_qkv_qk(1)
            emit_lp(n0)
            n1 = emit_attn(0, 1)
            emit_recip(n0)
            emit_qkv_v(1)
            emit_lp(n1)
            n2 = emit_attn(1, 0)
            emit_recip(n1)
            emit_finish(n0)
            emit_qkv_qk(2)
            emit_lp(n2)
            n3 = emit_attn(1, 1)
            emit_recip(n2)
            emit_finish(n1)
            emit_qkv_v(2)
            emit_lp(n3)
            emit_proj(0)
            n4 = emit_attn(2, 0)
            emit_recip(n3)
            emit_finish(n2)
            emit_qkv_qk(3)
            emit_lp(n4)
            n5 = emit_attn(2, 1)
            emit_recip(n4)
            emit_finish(n3)
            emit_qkv_v(3)
            emit_lp(n5)
            emit_proj(1)
            n6 = emit_attn(3, 0)
            emit_recip(n5)
            emit_finish(n4)
            emit_lp(n6)
            n7 = emit_attn(3, 1)
            emit_lp(n7)
            emit_recip(n6)
            emit_finish(n5)
            emit_proj(2, mts=[8, 9])
            emit_recip(n7)
            emit_finish(n6)
            emit_proj(2, mts=[10, 11])
            emit_finish(n7)
            emit_proj(3)

    nc.compile()
    return nc


def _in_maps(x, seg, Wqkv, Wproj, mask_arrs):
    maps = []
    for c in range(8):
        b, g = divmod(c, 4)
        h0 = g * 4
        cs, ce = h0 * 64, h0 * 64 + 256
        maps.append({
            "xT": np.ascontiguousarray(x[b].T).astype(nbf),
            "wqkv": np.ascontiguousarray(np.concatenate(
                [Wqkv[:, cs:ce], Wqkv[:, D + cs:D + ce], Wqkv[:, 2 * D + cs:2 * D + ce]],
                axis=1)).astype(nbf),
            "wp": np.ascontiguousarray(Wproj[cs:ce, :]).astype(nbf),
            "mask": mask_arrs[b],
        })
    return maps


_CACHE = {}


def _prepare(x, segment_ids, W_qkv, W_proj):
    x = np.asarray(x, np.float32)
    seg = np.asarray(segment_ids)
    Wqkv = np.asarray(W_qkv, np.float32)
    Wproj = np.asarray(W_proj, np.float32)
    tiles, mask_arrs, wtot = _schedule(seg)
    key = (tuple((qc, t) for qc in tiles for t in tiles[qc]), wtot)
    if key not in _CACHE:
        _CACHE[key] = _build(tiles, wtot)
    nc = _CACHE[key]
    return nc, _in_maps(x, seg, Wqkv, Wproj, mask_arrs)


def kernel(x, segment_ids, W_qkv, W_proj):
    nc, in_maps = _prepare(x, segment_ids, W_qkv, W_proj)
    res = bass_utils.run_bass_kernel_spmd(nc, in_maps, core_ids=list(range(8)))
    out = np.zeros((B, T, D), np.float32)
    for c in range(8):
        out[c // 4] += res.results[c]["out"].astype(np.float32)
    return out
